# revision 11
# baseline (speedup 1.0000x reference)
"""GroupDense kernel for Trainium2 (8 NeuronCores, SPMD data-parallel over batch).

y[b,s,g*64+v] = relu(sum_u x[b,s,g*64+u] * w[g,u,v])
x: [8, 2048, 4096] fp32, w: [64, 64, 64] fp32.

Per-core: core i processes batch i. Host pre-permutes/casts the shard to
x [P=128, CB=32, TOK=2048] bf16 (channel-within-block on partitions) so the
contraction dim lands on SBUF partitions with no on-chip transpose, and
multi-block chunks are DMA-contiguous per partition. Weights are packed into
32 block-diagonal [128,128] bf16 tiles (two 64x64 groups each), loaded in 4
chunks so the first matmul starts early. The matmul runs weight-stationary
(lhsT = w block, rhs = x streaming 512 tokens) producing y^T per block;
ReLU (split ACT/DVE across PSUM banks) writes bf16. Loads ride the SP HWDGE
ring, stores the ACT ring, with the last two stores on SP/SWDGE so all
queues drain the tail. Chunk sizes ramp 1,1,2,4,... so the store stream
starts early (both DMA rings saturate ~420 GB/s aggregate) while the bulk
uses few large DMAs. Host un-permutes y and upcasts to fp32. HBM traffic is
16 MB in + 16 MB out per core.
"""

import numpy as np
import ml_dtypes

import concourse.bass as bass
import concourse.mybir as mybir
import concourse.tile as tile
from concourse import bacc
from concourse.bass import ds, ts
from concourse.bass_utils import run_bass_kernel_spmd

B, S, C = 8, 2048, 4096
U = 64
G = C // U  # 64 groups
NCORES = 8
TOK = (B * S) // NCORES  # 2048 tokens per core
P = 128
CB = C // P  # 32 channel blocks (2 groups each)
NSEG = TOK // 512  # 4 matmul segments of 512 tokens per block
HALF = (NSEG // 2) * 512

# Per-block (J=1) DMAs pipeline best: finer flow control beats the larger
# descriptors of multi-block chunks (measured).
CHUNKS = [1] * CB
assert sum(CHUNKS) == CB

F32 = mybir.dt.float32
BF16 = mybir.dt.bfloat16
BF16NP = ml_dtypes.bfloat16

_cached_nc = None


def _build():
    global _cached_nc
    if _cached_nc is not None:
        return _cached_nc

    nc = bacc.Bacc("TRN2", target_bir_lowering=False)

    x_d = nc.dram_tensor("x", [P, CB, TOK], BF16, kind="ExternalInput")
    w_d = nc.dram_tensor("w2", [P, CB * P], BF16, kind="ExternalInput")
    y_d = nc.dram_tensor("y", [P, CB, TOK], BF16, kind="ExternalOutput")

    with tile.TileContext(nc) as tc:
        with (
            tc.tile_pool(name="wpool", bufs=1) as wpool,
            tc.tile_pool(name="xpool", bufs=10) as xpool,
            tc.tile_pool(name="ypool", bufs=6) as ypool,
            tc.tile_pool(name="ps", bufs=2, space="PSUM") as ps,
        ):
            # w in ramped chunks: a tiny first chunk unblocks the first
            # matmul ~1us earlier.
            w_s = wpool.tile([P, CB, P], BF16)
            WCHUNKS = [2, 6, 12, 12]
            wc0 = 0
            for wj in WCHUNKS:
                nc.scalar.dma_start(
                    w_s[:, ds(wc0, wj), :],
                    w_d[:, ds(wc0 * P, wj * P)],
                )
                wc0 += wj

            nchunks = len(CHUNKS)
            c0 = 0
            for ci, J in enumerate(CHUNKS):
                x_t = xpool.tile([P, J, TOK], BF16)
                # a few early loads ride the scalar ring (still idle before
                # the store stream ramps) so the read stream front-loads at
                # two-ring rate and finishes sooner.
                load_eng = nc.scalar if ci in (1, 3, 5, 7) else nc.sync
                load_eng.dma_start(x_t[:], x_d[:, ds(c0, J), :])

                y_t = ypool.tile([P, J, TOK], BF16)
                for j in range(J):
                    cb = c0 + j
                    pY = ps.tile([P, NSEG, 512], F32)
                    for i in range(NSEG):
                        nc.tensor.matmul(
                            pY[:, i, :],
                            w_s[:, cb, :],
                            x_t[:, j, ds(i * 512, 512)],
                            start=True,
                            stop=True,
                        )
                    nc.scalar.activation(
                        y_t[:, j, 0:HALF],
                        pY[:, 0 : NSEG // 2, :],
                        mybir.ActivationFunctionType.Relu,
                    )
                    nc.vector.tensor_scalar_max(
                        y_t[:, j, HALF:TOK], pY[:, NSEG // 2 : NSEG, :], 0.0
                    )
                # tail: alternate the last stores across both HWDGE rings so
                # they drain concurrently once loads are done (last on sync,
                # which frees up first).
                if ci >= nchunks - 6 and (nchunks - 1 - ci) % 2 == 0:
                    eng = nc.sync
                else:
                    eng = nc.scalar
                eng.dma_start(y_d[:, ds(c0, J), :], y_t[:])
                c0 += J

    nc.compile()
    _cached_nc = nc
    return nc


def _pack_weights(kern):
    # [P, CB*P] bf16: block-diagonal pairs, partition-major (u within block
    # on partitions; blocks x out-channel along the free dim).
    w2 = np.zeros((CB, P, P), dtype=np.float32)
    w2[:, :U, :U] = kern[0::2]
    w2[:, U:, U:] = kern[1::2]
    return np.ascontiguousarray(
        w2.transpose(1, 0, 2).reshape(P, CB * P).astype(BF16NP)
    )


def prep_inputs(x, kern):
    x = np.asarray(x, dtype=np.float32)
    w2 = _pack_weights(np.asarray(kern, dtype=np.float32))
    in_maps = []
    for i in range(NCORES):
        # [TOK, C] -> [P, CB, TOK]: x_h[p, cb, t] = x[t, cb*128 + p]
        xh = x[i].reshape(TOK, CB, P).transpose(2, 1, 0).astype(BF16NP)
        in_maps.append({"x": np.ascontiguousarray(xh), "w2": w2})
    return in_maps


def postprocess(res):
    out = np.empty((NCORES, TOK, C), dtype=np.float32)
    for i in range(NCORES):
        yh = np.asarray(res.results[i]["y"])  # [P, CB, TOK] bf16
        out[i] = yh.transpose(2, 1, 0).reshape(TOK, C).astype(np.float32)
    return np.ascontiguousarray(out.reshape(B, S, C))


def kernel(x, kernel):
    nc = _build()
    in_maps = prep_inputs(x, kernel)
    res = run_bass_kernel_spmd(nc, in_maps, list(range(NCORES)))
    return postprocess(res)


# revision 12
# speedup vs baseline: 1.0630x; 1.0630x over previous
"""GroupDense kernel for Trainium2 (8 NeuronCores, SPMD data-parallel over batch).

y[b,s,g*64+v] = relu(sum_u x[b,s,g*64+u] * w[g,u,v])
x: [8, 2048, 4096] fp32, w: [64, 64, 64] fp32.

Per-core: core i processes batch i. Host pre-transposes/casts the shard to
x^T [C, TOK] bf16 so the contraction dim lands on SBUF partitions with no
on-chip transpose, and packs weights into 32 block-diagonal [128,128] bf16
tiles (two 64x64 groups each). The matmul runs weight-stationary
(lhsT = w block, rhs = x^T streaming 512 tokens) so the output is y^T
[outch, tok]; ReLU (split across ACT and DVE) writes bf16, stores go out on
the ACT HWDGE ring while loads ride the SP ring. Host un-transposes y^T and
upcasts to fp32. HBM traffic is 16 MB in + 16 MB out per core.
"""

import numpy as np
import ml_dtypes

import concourse.bass as bass
import concourse.mybir as mybir
import concourse.tile as tile
from concourse import bacc
from concourse.bass import ds, ts
from concourse.bass_utils import run_bass_kernel_spmd

B, S, C = 8, 2048, 4096
U = 64
G = C // U  # 64 groups
NCORES = 8
TOK = (B * S) // NCORES  # 2048 tokens per core
P = 128
CB = C // P  # 32 channel blocks (2 groups each)
NSEG = TOK // 512  # 4 matmul segments of 512 tokens per stripe

F32 = mybir.dt.float32
BF16 = mybir.dt.bfloat16
BF16NP = ml_dtypes.bfloat16

_cached_nc = None


def _build():
    global _cached_nc
    if _cached_nc is not None:
        return _cached_nc

    nc = bacc.Bacc("TRN2", target_bir_lowering=False)

    x_d = nc.dram_tensor("x", [C, TOK], BF16, kind="ExternalInput")
    w_d = nc.dram_tensor("w2", [P, CB * P], BF16, kind="ExternalInput")
    y_d = nc.dram_tensor("y", [C, TOK], BF16, kind="ExternalOutput")

    with tile.TileContext(nc) as tc:
        with (
            tc.tile_pool(name="wpool", bufs=1) as wpool,
            tc.tile_pool(name="xpool", bufs=6) as xpool,
            tc.tile_pool(name="ypool", bufs=6) as ypool,
            tc.tile_pool(name="ps", bufs=2, space="PSUM") as ps,
        ):
            # w in 4 chunks so the first matmul (needs chunk 0 only) starts
            # ~6us earlier; that pulls the whole store stream left, keeping
            # both HWDGE rings streaming through the ramp.
            w_s = wpool.tile([P, CB, P], BF16)
            WBLK = CB // 4
            for c in range(4):
                nc.scalar.dma_start(
                    w_s[:, ds(c * WBLK, WBLK), :],
                    w_d[:, ds(c * WBLK * P, WBLK * P)],
                )

            for cb in range(CB):
                x_t = xpool.tile([P, TOK], BF16)
                nc.sync.dma_start(x_t[:], x_d[ts(cb, P), :])

                y_t = ypool.tile([P, TOK], BF16)
                pY = ps.tile([P, NSEG, 512], F32)
                for i in range(NSEG):
                    nc.tensor.matmul(
                        pY[:, i, :],
                        w_s[:, cb, :],
                        x_t[:, ds(i * 512, 512)],
                        start=True,
                        stop=True,
                    )
                half = (NSEG // 2) * 512
                nc.scalar.activation(
                    y_t[:, 0:half],
                    pY[:, 0 : NSEG // 2, :],
                    mybir.ActivationFunctionType.Relu,
                )
                nc.vector.tensor_scalar_max(
                    y_t[:, half:TOK], pY[:, NSEG // 2 : NSEG, :], 0.0
                )
                # tail: once loads are done the sync ring idles — drain the
                # last stores across both rings.
                store_eng = nc.sync if (cb >= CB - 6 and cb % 2 == 0) else nc.scalar
                store_eng.dma_start(y_d[ts(cb, P), :], y_t[:])

    nc.compile()
    _cached_nc = nc
    return nc


def _pack_weights(kern):
    # [P, CB*P] bf16: block-diagonal pairs, partition-major (u within block
    # on partitions; blocks x out-channel along the free dim).
    w2 = np.zeros((CB, P, P), dtype=np.float32)
    w2[:, :U, :U] = kern[0::2]
    w2[:, U:, U:] = kern[1::2]
    return np.ascontiguousarray(
        w2.transpose(1, 0, 2).reshape(P, CB * P).astype(BF16NP)
    )


def prep_inputs(x, kern):
    x = np.asarray(x, dtype=np.float32)
    w2 = _pack_weights(np.asarray(kern, dtype=np.float32))
    in_maps = [
        {
            "x": np.ascontiguousarray(x[i].reshape(TOK, C).T.astype(BF16NP)),
            "w2": w2,
        }
        for i in range(NCORES)
    ]
    return in_maps


def postprocess(res):
    y = np.stack(
        [
            np.asarray(res.results[i]["y"]).astype(np.float32).T
            for i in range(NCORES)
        ],
        axis=0,
    )
    return np.ascontiguousarray(y.reshape(B, S, C))


def kernel(x, kernel):
    nc = _build()
    in_maps = prep_inputs(x, kernel)
    res = run_bass_kernel_spmd(nc, in_maps, list(range(NCORES)))
    return postprocess(res)
